# revision 22
# baseline (speedup 1.0000x reference)
"""Trainium2 Bass kernel for nn_BipartiteRGAT (2-layer relational GAT).

Strategy (8 NeuronCores, SPMD, v2):
  - Nodes interleave-sharded: core c owns rows [c*1250,(c+1)*1250) of each of
    the 3 omics (3750 nodes/core, block-permuted node order).
  - Host pre-transposes x blocks to feature-major bf16; projections are
    direct W @ x^T bf16 matmuls (512-wide moving operand), AllGather x^T bf16.
  - Layer 1: every core builds the full per-relation table
    T1a[r] = [x@W1[r] | x@(W1[r]k1)] (132 bf16 cols in 512B rows) with
    3-relation-wide bf16 matmuls; per-dst-core edges sorted into
    (dst-window, relation)-pure 128-edge chunks; ONE dma_gather per edge
    (value+K in one 512B row); Q terms come from an SBUF per-window table
    via a host-precomputed one-hot matmul (no gather); messages scatter-add
    into PSUM via one-hot matmuls using host-precomputed S matrices.
  - Layer 2 tables are compacted to the (core, relation)-used source rows
    (~12% of dense), built from gathered layer-1 rows.
  - Segment softmax is dst-local so no cross-core reduction; collectives are
    two bf16 AllGathers.
"""
import sys

import numpy as np
import ml_dtypes

sys.path.insert(0, "/opt/trn_rl_repo")

import concourse.bass as bass  # noqa: F401
import concourse.bacc as bacc
import concourse.tile as tile
import concourse.mybir as mybir

F32 = mybir.dt.float32
BF16 = mybir.dt.bfloat16
I16 = mybir.dt.int16
AF = mybir.ActivationFunctionType
ALU = mybir.AluOpType
NPBF = ml_dtypes.bfloat16

N_CORES = 8
N = 30000
NO = 10000
SEGN = 1250   # per-omic rows per core
NPC = 3750    # nodes per core (block size)
R = 6
PROJ = 256
H1, C1, HC1 = 4, 32, 128
C2 = 128
LABELS = 5
E = 400000
TPAD = 30720  # padded dense table rows

GMAX_CH = 8   # chunks per dma_gather segment (1024 idxs)
PHASES = 4    # debug: 0=null 1=proj+AG 2=+L1 table 3=+L1 edges+AG 4=full
SCRATCH = 32768
NQUEUES = 1

L1_WIN = [128] * 29 + [38]
L1_MBS = [list(range(5 * i, 5 * i + 5)) for i in range(6)]
L2_WIN = [128] * 9 + [98]
L2_MBS = [list(range(5 * i, 5 * i + 5)) for i in range(2)]

CN1 = HC1 + H1          # 132 table cols for L1
CN2 = C2 + 1            # 129 table cols for L2


def _perm():
    g = np.arange(N)
    s = g // NO
    i = g % NO
    c = i // SEGN
    return c * NPC + s * SEGN + (i % SEGN)


def _wrap_idx(slots):
    s = np.asarray(slots, np.int64)
    assert s.max(initial=0) < 32768
    w = s.reshape(-1, 16).T.astype(np.int16)
    return np.ascontiguousarray(np.tile(w, (8, 1)))


class _LayerPlan:
    pass


def _plan_layer(pdst, psrc, et, win_sizes, mbs):
    """Shared chunk layout + per-core slot arrays for one layer's edges."""
    nwin = len(win_sizes)
    core = pdst // NPC
    loc = pdst % NPC
    w = loc // 128
    dstl = loc % 128

    cnt = np.zeros((N_CORES, nwin, R), np.int64)
    np.add.at(cnt, (core, w, et), 1)
    K = np.maximum(1, -(-cnt.max(axis=0) // 128))  # [nwin, R]

    chunk_meta = []          # chunk id -> (r, w)
    gbase = np.zeros((R, nwin), np.int64)  # (r, w) -> first slot
    segs = []                # (chunk_off, nch, r, mb_idx)
    for mi, mb in enumerate(mbs):
        for r in range(R):
            g_start = len(chunk_meta)
            for wi in mb:
                gbase[r, wi] = len(chunk_meta) * 128
                for _ in range(K[wi, r]):
                    chunk_meta.append((r, wi))
            nch = len(chunk_meta) - g_start
            off = g_start
            while nch > 0:
                take = min(GMAX_CH, nch)
                segs.append((off, take, r, mi))
                off += take
                nch -= take
    nchunks = len(chunk_meta)
    nslots = nchunks * 128

    p = _LayerPlan()
    p.K, p.chunk_meta, p.segs = K, chunk_meta, segs
    p.nchunks, p.nslots, p.mbs = nchunks, nslots, mbs
    p.win_sizes = win_sizes
    p.s_src, p.s_real, p.spair = [], [], []
    chunk_r = np.array([m[0] for m in chunk_meta], np.int64)
    for c in range(N_CORES):
        m = core == c
        e_r, e_w, e_dl = et[m], w[m], dstl[m]
        e_src = psrc[m]
        order = np.lexsort((e_w, e_r))
        e_r, e_w, e_dl = e_r[order], e_w[order], e_dl[order]
        e_src = e_src[order]
        key = e_r * nwin + e_w
        start = np.searchsorted(key, np.arange(R * nwin), side="left")
        rank = np.arange(key.size) - start[key]
        slot = gbase[e_r, e_w] + rank
        s_src = np.zeros(nslots, np.int64)
        s_real = np.zeros(nslots, bool)
        s_src[slot] = e_src
        s_real[slot] = True
        # one-hot pair [S | S^T] per chunk
        A = np.zeros((nchunks, 128, 128), np.float32)
        A[slot // 128, slot % 128, e_dl] = 1.0
        sp = np.stack([A, A.transpose(0, 2, 1)], axis=2)  # [c, p, 2, x]
        sp = np.ascontiguousarray(
            sp.transpose(1, 0, 2, 3)).astype(NPBF)       # [128, c, 2, 128]
        p.s_src.append(s_src)
        p.s_real.append(s_real)
        p.spair.append(sp)
    p.chunk_r = chunk_r
    return p


def _ag1_rowid(m):
    c2 = m // NPC
    ll = m % NPC
    return (c2 * 128 + ll % 128) * 30 + ll // 128


def _host_prep(inputs):
    perm = _perm()
    ei = np.asarray(inputs["edge_index"]).astype(np.int64)
    et = np.asarray(inputs["edge_type"]).astype(np.int64)
    src, dst = ei[0], ei[1]
    psrc, pdst = perm[src], perm[dst]

    plan1 = _plan_layer(pdst, psrc, et, L1_WIN, L1_MBS)
    m2 = dst < NO
    plan2 = _plan_layer(pdst[m2], psrc[m2], et[m2], L2_WIN, L2_MBS)

    # ---- layer-2 compact tables: per (core, r) used source rows ----
    rows2, g1i_l2 = [], []
    max_rows = 1
    for c in range(N_CORES):
        s_src, s_real = plan2.s_src[c], plan2.s_real[c]
        s_r = plan2.chunk_r[np.arange(plan2.nslots) // 128]
        rr, remap = [], np.zeros(plan2.nslots, np.int64)
        for r in range(R):
            mrr = s_real & (s_r == r)
            rows = np.unique(s_src[mrr])
            rr.append(rows)
            max_rows = max(max_rows, len(rows))
            remap[mrr] = np.searchsorted(rows, s_src[mrr])
        rows2.append(rr)
        g1i_l2.append(remap)
    NROWS2 = 128 * (-(-max_rows // 128))
    NT2 = NROWS2 // 128
    xg2i = []
    for c in range(N_CORES):
        lst = []
        for r in range(R):
            rows = rows2[c][r]
            padded = np.zeros(NROWS2, np.int64)
            padded[: len(rows)] = _ag1_rowid(rows)
            lst.append(padded)
        xg2i.append(_wrap_idx(np.concatenate(lst)))

    W1 = np.asarray(inputs["W1"], np.float64)
    q1 = np.asarray(inputs["q1"], np.float64)
    k1 = np.asarray(inputs["k1"], np.float64)
    W2 = np.asarray(inputs["W2"], np.float64)
    q2 = np.asarray(inputs["q2"], np.float64)
    k2 = np.asarray(inputs["k2"], np.float64)

    # L1: [W1[r] | W1[r]@k1] grouped 3 relations -> [2, 256, 396]
    cw1g = np.stack([
        np.concatenate(
            [np.concatenate([W1[r], W1[r] @ k1], axis=1)
             for r in range(3 * g, 3 * g + 3)], axis=1)
        for g in range(2)])
    cw1 = np.ascontiguousarray(
        cw1g.reshape(2, 2, 128, 3 * CN1).transpose(2, 1, 0, 3)).astype(NPBF)
    cwq1 = np.ascontiguousarray(
        np.concatenate([W1[r] @ q1 for r in range(R)], axis=1)
        .reshape(2, 128, R * H1).transpose(1, 0, 2)).astype(NPBF)
    # L2: per-relation [W2[r] | W2[r]@k2] -> [128, 6, 129]
    cw2 = np.ascontiguousarray(
        np.stack([np.concatenate([W2[r], W2[r] @ k2], axis=1)
                  for r in range(R)], axis=1)).astype(NPBF)
    cwq2 = np.ascontiguousarray(
        np.concatenate([W2[r] @ q2 for r in range(R)], axis=1)).astype(NPBF)

    weights = {
        "WpT0": np.asarray(inputs["Wp0"], np.float32).T.astype(NPBF),
        "WpT1": np.asarray(inputs["Wp1"], np.float32).T.astype(NPBF),
        "WpT2": np.asarray(inputs["Wp2"], np.float32).T.astype(NPBF),
        "bp0": np.asarray(inputs["bp0"], np.float32).reshape(2, 128).T.copy(),
        "bp1": np.asarray(inputs["bp1"], np.float32).reshape(2, 128).T.copy(),
        "bp2": np.asarray(inputs["bp2"], np.float32).reshape(2, 128).T.copy(),
        "cw1": cw1, "cwq1": cwq1, "cw2": cw2, "cwq2": cwq2,
        "b1rep": np.broadcast_to(
            np.asarray(inputs["b1"], np.float32), (128, HC1)).copy(),
        "b2rep": np.broadcast_to(
            np.asarray(inputs["b2"], np.float32), (128, C2)).copy(),
        "blrep": np.broadcast_to(
            np.asarray(inputs["bl"], np.float32), (128, LABELS)).copy(),
        "WlT": np.ascontiguousarray(np.asarray(inputs["Wl"], np.float32).T),
        "identf": np.eye(128, dtype=np.float32),
        "identb": np.eye(128, dtype=np.float32).astype(NPBF),
    }

    x0 = np.asarray(inputs["x0"], np.float32)
    x1 = np.asarray(inputs["x1"], np.float32)
    x2 = np.asarray(inputs["x2"], np.float32)
    per_core = []
    for c in range(N_CORES):
        sl = slice(c * SEGN, (c + 1) * SEGN)
        d = {
            "xT0": np.ascontiguousarray(x0[sl].T).astype(NPBF),
            "xT1i": np.ascontiguousarray(x1[sl].T).astype(NPBF),
            "xT2i": np.ascontiguousarray(x2[sl].T).astype(NPBF),
            "g1i_l1": _wrap_idx(plan1.s_src[c]),
            "sp1": plan1.spair[c],
            "g1i_l2": _wrap_idx(g1i_l2[c]),
            "sp2": plan2.spair[c],
            "xg2i": xg2i[c],
        }
        d.update(weights)
        per_core.append(d)
    plan2.NROWS2, plan2.NT2 = NROWS2, NT2
    return plan1, plan2, per_core


# ---------------------------------------------------------------------------
# device program
# ---------------------------------------------------------------------------

def _build(plan1, plan2):
    nc = bacc.Bacc(
        "TRN2", target_bir_lowering=False, debug=False, num_devices=N_CORES,
        dynamic_dma_scratch_size=SCRATCH, num_swdge_queues=NQUEUES,
    )
    dt = nc.dram_tensor
    inp = {}
    NT2 = plan2.NT2
    for nm, shape, d in [
        ("xT0", [2000, SEGN], BF16), ("xT1i", [1500, SEGN], BF16),
        ("xT2i", [1000, SEGN], BF16),
        ("WpT0", [2000, PROJ], BF16), ("WpT1", [1500, PROJ], BF16),
        ("WpT2", [1000, PROJ], BF16),
        ("bp0", [128, 2], F32), ("bp1", [128, 2], F32), ("bp2", [128, 2], F32),
        ("cw1", [128, 2, 2, 3 * CN1], BF16), ("cwq1", [128, 2, R * H1], BF16),
        ("cw2", [128, R, CN2], BF16), ("cwq2", [128, R], BF16),
        ("b1rep", [128, HC1], F32), ("b2rep", [128, C2], F32),
        ("blrep", [128, LABELS], F32), ("WlT", [C2, LABELS], F32),
        ("identf", [128, 128], F32), ("identb", [128, 128], BF16),
        ("g1i_l1", [128, plan1.nslots // 16], I16),
        ("sp1", [128, plan1.nchunks, 2, 128], BF16),
        ("g1i_l2", [128, plan2.nslots // 16], I16),
        ("sp2", [128, plan2.nchunks, 2, 128], BF16),
        ("xg2i", [128, R * NT2 * 8], I16),
    ]:
        inp[nm] = dt(nm, shape, d, kind="ExternalInput").ap()
    out_c = dt("out_c", [1280, LABELS], F32, kind="ExternalOutput").ap()

    with tile.TileContext(nc) as tc:
        _emit(nc, tc, inp, out_c, plan1, plan2)
    nc.compile()
    return nc


def _emit(nc, tc, inp, out_c, plan1, plan2):
    import contextlib

    ctx = contextlib.ExitStack()
    with ctx:
        dram = ctx.enter_context(tc.tile_pool(name="dram", bufs=1, space="DRAM"))
        const = ctx.enter_context(tc.tile_pool(name="const", bufs=1))

        if PHASES == 0:
            with tc.tile_pool(name="dbg", bufs=1) as dbg:
                td = dbg.tile([128, LABELS], F32)
                nc.vector.memset(td[:], 0.0)
                nc.sync.dma_start(out_c[0:128], td[:])
            return

        T1a = [dram.tile([TPAD, 256], BF16, name=f"T1a{r}") for r in range(R)]
        T1b = [dram.tile([plan2.NROWS2, 256], BF16, name=f"T1b{r}")
               for r in range(R)]
        agx_in = dram.tile([128, 2 * NPC], BF16)
        agx_out = nc.dram_tensor(
            "agx_out_sh", [N_CORES * 128, 2 * NPC], BF16,
            addr_space="Shared").ap()
        ag1_in = dram.tile([128, 30 * 128], BF16)
        ag1_out = nc.dram_tensor(
            "ag1_out_sh", [TPAD, 128], BF16, addr_space="Shared").ap()

        t_identf = const.tile([128, 128], F32)
        t_identb = const.tile([128, 128], BF16)
        t_b1 = const.tile([128, HC1], F32)
        t_b2 = const.tile([128, C2], F32)
        t_bl = const.tile([128, LABELS], F32)
        t_WlT = const.tile([C2, LABELS], F32)
        t_cw1 = const.tile([128, 2, 2, 3 * CN1], BF16)
        t_cwq1 = const.tile([128, 2, R * H1], BF16)
        t_cw2 = const.tile([128, R, CN2], BF16)
        t_cwq2 = const.tile([128, R], BF16)
        qtab1 = const.tile([128, 30, R * H1], BF16)
        qtab2 = const.tile([128, 10, R], BF16)
        xrow = const.tile([128, 30, HC1], BF16)
        for t, k in [(t_identf, "identf"), (t_identb, "identb"),
                     (t_b1, "b1rep"), (t_b2, "b2rep"), (t_bl, "blrep"),
                     (t_WlT, "WlT"), (t_cw1, "cw1"), (t_cwq1, "cwq1"),
                     (t_cw2, "cw2"), (t_cwq2, "cwq2")]:
            nc.sync.dma_start(t[:], inp[k][:])
        nc.vector.memset(xrow[:], 0.0)
        nc.vector.memset(qtab1[:], 0.0)
        nc.vector.memset(qtab2[:], 0.0)

        # ================= phase P: projections -> x^T (bf16) ==============
        pp1_cm = tc.tile_pool(name="pp1", bufs=1)
        pp2_cm = tc.tile_pool(name="pp2", bufs=2)
        pps_cm = tc.tile_pool(name="pps", bufs=4, space="PSUM")
        pp1 = pp1_cm.__enter__()
        pp2 = pp2_cm.__enter__()
        pps = pps_cm.__enter__()
        xT = pp1.tile([128, 2, NPC], BF16, tag="xTproj")
        for s, (xk, wk, bk, isz) in enumerate([
            ("xT0", "WpT0", "bp0", 2000),
            ("xT1i", "WpT1", "bp1", 1500),
            ("xT2i", "WpT2", "bp2", 1000),
        ]):
            nk = -(-isz // 128)
            lastk = isz - (nk - 1) * 128
            t_w = pp2.tile([128, nk, PROJ], BF16, tag="wpt")
            t_x = pp2.tile([128, nk, SEGN], BF16, tag="xin")
            nh = nk // 2
            for a, b in [(0, nh), (nh, nk - 1)]:
                nc.sync.dma_start(
                    t_w[:, a:b, :],
                    inp[wk][a * 128: b * 128].rearrange(
                        "(k p) o -> p k o", p=128))
                nc.scalar.dma_start(
                    t_x[:, a:b, :],
                    inp[xk][a * 128: b * 128].rearrange(
                        "(k p) n -> p k n", p=128))
            nc.sync.dma_start(t_w[:lastk, nk - 1, :], inp[wk][(nk - 1) * 128:])
            nc.scalar.dma_start(t_x[:lastk, nk - 1, :], inp[xk][(nk - 1) * 128:])
            t_bp = pp1.tile([128, 2], F32, tag="bp")
            nc.sync.dma_start(t_bp[:], inp[bk][:])
            col0 = s * SEGN
            for g0, nn in [(0, 512), (512, 512), (1024, 226)]:
                for oh in range(2):
                    p_p = pps.tile([128, 512], F32, tag="mm")
                    for kk in range(nk):
                        kw = 128 if kk < nk - 1 else lastk
                        nc.tensor.matmul(
                            p_p[:, :nn],
                            t_w[:kw, kk, oh * 128: oh * 128 + 128],
                            t_x[:kw, kk, g0: g0 + nn],
                            start=(kk == 0), stop=(kk == nk - 1))
                    nc.scalar.activation(
                        xT[:, oh, col0 + g0: col0 + g0 + nn], p_p[:, :nn],
                        AF.Relu, bias=t_bp[:, oh: oh + 1])
        # Qtab1: q-projections of my nodes per (window, relation)
        for nt in range(30):
            nn = 128 if nt < 29 else NPC - 29 * 128
            p_q = pps.tile([128, R * H1], F32, tag="qt")
            for mt in range(2):
                nc.tensor.matmul(
                    p_q[:nn, :], xT[:, mt, nt * 128: nt * 128 + nn],
                    t_cwq1[:, mt, :], start=(mt == 0), stop=(mt == 1))
            nc.vector.tensor_copy(qtab1[:nn, nt, :], p_q[:nn, :])
        nc.sync.dma_start(agx_in[:], xT[:].rearrange("p m n -> p (m n)"))
        nc.gpsimd.collective_compute(
            "AllGather", ALU.bypass,
            replica_groups=[list(range(N_CORES))],
            ins=[agx_in.opt()], outs=[agx_out[:]])
        pps_cm.__exit__(None, None, None)
        pp2_cm.__exit__(None, None, None)
        pp1_cm.__exit__(None, None, None)

        if PHASES == 1:
            with tc.tile_pool(name="dbg", bufs=1) as dbg:
                td = dbg.tile([128, LABELS], F32)
                tb = dbg.tile([128, LABELS], BF16)
                nc.sync.dma_start(tb[:], agx_out[0:128, 0:LABELS])
                nc.vector.tensor_copy(td[:], tb[:])
                nc.sync.dma_start(out_c[0:128], td[:])
            return

        sb1 = ctx.enter_context(tc.tile_pool(name="sb1", bufs=1))
        sb2 = ctx.enter_context(tc.tile_pool(name="sb2", bufs=2))
        sb3 = ctx.enter_context(tc.tile_pool(name="sb3", bufs=4))
        pools = dict(sb1=sb1, sb2=sb2, sb3=sb3, tc=tc)

        # ================= L1 dense table build ============================
        psm_cm = tc.tile_pool(name="psma", bufs=4, space="PSUM")
        ps_m = psm_cm.__enter__()
        for rb in range(N_CORES):
            blk = sb2.tile([128, 2 * NPC], BF16, tag="xblk")
            nc.sync.dma_start(blk[:], agx_out[rb * 128: (rb + 1) * 128, :])
            blk3 = blk[:].rearrange("p (m n) -> p m n", m=2)
            for g in range(3):
                st = sb2.tile([128, 10, 2, 3 * CN1], BF16, tag="st")
                for ntl in range(10):
                    nt = g * 10 + ntl
                    nn = 128 if nt < 29 else NPC - 29 * 128
                    for g3 in range(2):
                        p_t = ps_m.tile([128, 3 * CN1], F32, tag="mm")
                        for mt in range(2):
                            nc.tensor.matmul(
                                p_t[:nn, :],
                                blk3[:, mt, nt * 128: nt * 128 + nn],
                                t_cw1[:, mt, g3, :],
                                start=(mt == 0), stop=(mt == 1))
                        if (ntl + g3) % 2 == 0:
                            nc.scalar.copy(st[:nn, ntl, g3, :], p_t[:nn, :])
                        else:
                            nc.vector.tensor_copy(
                                st[:nn, ntl, g3, :], p_t[:nn, :])
                r0 = rb * NPC + g * 1280
                nfull = 10 if g < 2 else 9
                for r in range(R):
                    co = (r % 3) * CN1
                    nc.sync.dma_start(
                        T1a[r][r0: r0 + nfull * 128, 0:CN1].rearrange(
                            "(t p) c -> p t c", p=128),
                        st[:, :nfull, r // 3, co: co + CN1])
                    if nfull < 10:
                        nc.sync.dma_start(
                            T1a[r][r0 + 1152: r0 + 1190, 0:CN1],
                            st[:38, 9, r // 3, co: co + CN1])
        psm_cm.__exit__(None, None, None)

        if PHASES == 2:
            with tc.tile_pool(name="dbg", bufs=1) as dbg:
                tdb = dbg.tile([128, LABELS], BF16)
                td = dbg.tile([128, LABELS], F32)
                nc.sync.dma_start(tdb[:], T1a[0].opt()[0:128, 0:LABELS])
                nc.vector.tensor_copy(td[:], tdb[:])
                nc.sync.dma_start(out_c[0:128], td[:])
            return

        # ================= L1 edge phase ==================================
        _edges(nc, pools, plan=plan1, g1i=inp["g1i_l1"], sp_in=inp["sp1"],
               qtab=qtab1, T=T1a, heads=H1, chead=C1, hc=HC1,
               bias_rep=t_b1, lname="a",
               out_xT=("xrow", xrow))
        nc.sync.dma_start(
            ag1_in[:], xrow[:].rearrange("p w c -> p (w c)"))
        nc.gpsimd.collective_compute(
            "AllGather", ALU.bypass,
            replica_groups=[list(range(N_CORES))],
            ins=[ag1_in.opt()], outs=[ag1_out[:]])

        if PHASES == 3:
            with tc.tile_pool(name="dbg", bufs=1) as dbg:
                tdb = dbg.tile([128, LABELS], BF16)
                td = dbg.tile([128, LABELS], F32)
                nc.sync.dma_start(tdb[:], ag1_out[0:128, 0:LABELS])
                nc.vector.tensor_copy(td[:], tdb[:])
                nc.sync.dma_start(out_c[0:128], td[:])
            return

        # ================= L2 compact table build ==========================
        NT2 = plan2.NT2
        psb_cm = tc.tile_pool(name="psmb", bufs=2, space="PSUM")
        ps_b = psb_cm.__enter__()
        t_xg2i = sb1.tile([128, R * NT2 * 8], I16, tag="xg2i")
        nc.sync.dma_start(t_xg2i[:], inp["xg2i"][:])
        GC = GMAX_CH
        for r in range(R):
            for gg in range(-(-NT2 // GC)):
                ntg = min(GC, NT2 - gg * GC)
                xg = sb2.tile([128, GC, 128], BF16, tag="xg")
                i0 = (r * NT2 + gg * GC) * 8
                nc.gpsimd.dma_gather(
                    xg[:, :ntg, :], ag1_out[:],
                    t_xg2i[:, i0: i0 + ntg * 8],
                    ntg * 128, ntg * 128, 128)
                st2 = sb2.tile([128, GC, CN2], BF16, tag="st2")
                for k in range(ntg):
                    p_tr = ps_b.tile([128, 128], BF16, tag="tr")
                    nc.tensor.transpose(
                        p_tr[:], xg[:, k, :], t_identb[:])
                    lhs = sb2.tile([128, 128], BF16, tag="lhs")
                    nc.vector.tensor_copy(lhs[:], p_tr[:])
                    p_2 = ps_b.tile([128, CN2], F32, tag="m2")
                    nc.tensor.matmul(
                        p_2[:], lhs[:], t_cw2[:, r, :], start=True, stop=True)
                    nc.scalar.copy(st2[:, k, :], p_2[:])
                nc.sync.dma_start(
                    T1b[r][gg * GC * 128: gg * GC * 128 + ntg * 128, 0:CN2]
                    .rearrange("(t p) c -> p t c", p=128),
                    st2[:, :ntg, :])
        # Qtab2 from my omic-0 rows
        for nt in range(10):
            nn = L2_WIN[nt]
            p_tr = ps_b.tile([128, 128], BF16, tag="tr")
            nc.tensor.transpose(
                p_tr[:, :nn], xrow[:nn, nt, :], t_identb[:nn, :nn])
            lhs = sb2.tile([128, 128], BF16, tag="lhs")
            nc.vector.tensor_copy(lhs[:, :nn], p_tr[:, :nn])
            p_q = ps_b.tile([128, R], F32, tag="q2")
            nc.tensor.matmul(
                p_q[:nn, :], lhs[:, :nn], t_cwq2[:], start=True, stop=True)
            nc.vector.tensor_copy(qtab2[:nn, nt, :], p_q[:nn, :])
        psb_cm.__exit__(None, None, None)

        # ================= L2 edge phase + head ===========================
        _edges(nc, pools, plan=plan2, g1i=inp["g1i_l2"], sp_in=inp["sp2"],
               qtab=qtab2, T=T1b, heads=1, chead=C2, hc=C2,
               bias_rep=t_b2, lname="b",
               out_xT=("final", (out_c, t_WlT, t_bl, t_identf)))
        with tc.tile_pool(name="zpad", bufs=1) as zp:
            tz = zp.tile([128, LABELS], F32)
            nc.vector.memset(tz[:], 0.0)
            nc.sync.dma_start(out_c[1250:1280], tz[:30])


def _edges(nc, pools, *, plan, g1i, sp_in, qtab, T, heads, chead, hc,
           bias_rep, lname, out_xT):
    sb1, sb3 = pools["sb1"], pools["sb3"]
    tc = pools["tc"]

    is_final = out_xT[0] == "final"
    psw_cm = tc.tile_pool(name=f"psw{lname}", bufs=1, space="PSUM")
    ps_w = psw_cm.__enter__()
    psq_cm = tc.tile_pool(
        name=f"psq{lname}", bufs=(1 if is_final else 2), space="PSUM")
    ps_q = psq_cm.__enter__()
    psf_cm = psf = None
    if is_final:
        psf_cm = tc.tile_pool(name=f"psf{lname}", bufs=1, space="PSUM")
        psf = psf_cm.__enter__()
    t_g1i = sb1.tile([128, plan.nslots // 16], I16, tag=f"g1i{lname}")
    nc.sync.dma_start(t_g1i[:], g1i[:])

    segs_by_mb = {}
    for (off, nch, r, mi) in plan.segs:
        segs_by_mb.setdefault(mi, []).append((off, nch, r))
    win_first, win_last = {}, {}
    for ci, (r, wi) in enumerate(plan.chunk_meta):
        win_first.setdefault(wi, ci)
        win_last[wi] = ci

    for mi, mb in enumerate(plan.mbs):
        wpsum = {
            wi: ps_w.tile([128, heads + hc], F32, tag=f"win{j}",
                          name=f"win{lname}{mi}w{j}")
            for j, wi in enumerate(mb)
        }
        for (off, nch, r) in segs_by_mb[mi]:
            g1 = sb3.tile([128, GMAX_CH, 256], BF16, tag="g1buf")
            nidx = nch * 128
            i0 = off * 8
            nc.gpsimd.dma_gather(
                g1[:, :nch, :], T[r].opt(), t_g1i[:, i0: i0 + nch * 8],
                nidx, nidx, 256)
            sp = sb3.tile([128, GMAX_CH, 2, 128], BF16, tag="spbuf")
            nc.sync.dma_start(sp[:, :nch], sp_in[:, off: off + nch])
            psq = ps_q.tile([128, GMAX_CH, heads], F32, tag="psq")
            for k in range(nch):
                ci = off + k
                _, wi = plan.chunk_meta[ci]
                # start only on k==0: `start` clears has_written for the
                # WHOLE bank, which would wipe earlier chunks' qi columns.
                nc.tensor.matmul(
                    psq[:, k, :], sp[:, k, 1, :],
                    qtab[:, wi, r * heads: (r + 1) * heads],
                    start=(k == 0), stop=(k == nch - 1))
            zq = sb3.tile([128, GMAX_CH, heads], F32, tag="zbuf")
            nc.vector.tensor_tensor(
                zq[:, :nch, :], psq[:, :nch, :],
                g1[:, :nch, hc: hc + heads], ALU.add)
            nc.vector.scalar_tensor_tensor(
                zq[:, :nch], zq[:, :nch], 0.2, zq[:, :nch],
                ALU.mult, ALU.max)
            pb = sb3.tile([128, GMAX_CH, heads], BF16, tag="pbuf")
            nc.scalar.activation(pb[:, :nch], zq[:, :nch], AF.Exp)
            Ms = sb3.tile([128, GMAX_CH, heads + hc], BF16, tag="M")
            nc.scalar.copy(Ms[:, :nch, :heads], pb[:, :nch, :])
            nc.vector.tensor_tensor(
                Ms[:, :nch, heads:].rearrange(
                    "p n (h c) -> p n h c", h=heads),
                pb[:, :nch, :].unsqueeze(-1).to_broadcast(
                    (128, nch, heads, chead)),
                g1[:, :nch, :hc].rearrange("p n (h c) -> p n h c", h=heads),
                ALU.mult)
            for k in range(nch):
                ci = off + k
                _, wi = plan.chunk_meta[ci]
                nc.tensor.matmul(
                    wpsum[wi][:], sp[:, k, 0, :], Ms[:, k, :],
                    start=(ci == win_first[wi]),
                    stop=(ci == win_last[wi]))
        for wi in mb:
            nn = plan.win_sizes[wi]
            psum = wpsum[wi]
            rs = sb3.tile([128, heads], F32, tag="rs")
            nc.vector.tensor_scalar_add(rs[:nn], psum[:nn, :heads], 1e-16)
            nc.vector.reciprocal(rs[:nn], rs[:nn])
            v = sb3.tile([128, hc], F32, tag="v")
            nc.vector.tensor_tensor(
                v[:nn].rearrange("p (h c) -> p h c", h=heads),
                psum[:nn, heads:].rearrange("p (h c) -> p h c", h=heads),
                rs[:nn].unsqueeze(-1).to_broadcast((nn, heads, chead)),
                ALU.mult)
            nc.vector.tensor_tensor(v[:nn], v[:nn], bias_rep[:nn], ALU.add)
            if out_xT[0] == "xrow":
                nc.scalar.activation(
                    out_xT[1][:nn, wi, :], v[:nn], AF.Relu)
            else:
                out_c, t_WlT, t_bl, t_identf = out_xT[1]
                nc.scalar.activation(v[:nn], v[:nn], AF.Relu)
                p_tr = psf.tile([128, 128], F32, tag="tp")
                nc.tensor.transpose(
                    p_tr[:, :nn], v[:nn, :], t_identf[:nn, :nn])
                xt = sb3.tile([128, 128], F32, tag="x2t")
                nc.vector.tensor_copy(xt[:, :nn], p_tr[:, :nn])
                p_f = psf.tile([128, LABELS], F32, tag="pf")
                nc.tensor.matmul(
                    p_f[:nn, :], xt[:, :nn], t_WlT[:], start=True, stop=True)
                ot = sb3.tile([128, LABELS], F32, tag="ot")
                nc.vector.tensor_tensor(
                    ot[:nn], p_f[:nn, :], t_bl[:nn], ALU.add)
                nc.sync.dma_start(out_c[wi * 128: wi * 128 + nn], ot[:nn])
    if psf_cm is not None:
        psf_cm.__exit__(None, None, None)
    psq_cm.__exit__(None, None, None)
    psw_cm.__exit__(None, None, None)


# ---------------------------------------------------------------------------
# host runner (persistent-jit SPMD executor via axon/PJRT)
# ---------------------------------------------------------------------------


class _Runner:
    def __init__(self, nc, n_cores):
        import jax
        from jax.sharding import Mesh, PartitionSpec, NamedSharding
        from jax.experimental.shard_map import shard_map
        from concourse.bass2jax import (
            _bass_exec_p, install_neuronx_cc_hook, partition_id_tensor)

        install_neuronx_cc_hook()
        self.jax = jax
        self.n_cores = n_cores
        partition_name = (
            nc.partition_id_tensor.name if nc.partition_id_tensor else None)
        in_names, out_names, out_avals, zero_outs = [], [], [], []
        for alloc in nc.m.functions[0].allocations:
            if not isinstance(alloc, mybir.MemoryLocationSet):
                continue
            name = alloc.memorylocations[0].name
            if alloc.kind == "ExternalInput":
                if name != partition_name:
                    in_names.append(name)
            elif alloc.kind == "ExternalOutput":
                out_names.append(name)
                shape = tuple(alloc.tensor_shape)
                dtype = mybir.dt.np(alloc.dtype)
                out_avals.append(jax.core.ShapedArray(shape, dtype))
                zero_outs.append(np.zeros(shape, dtype))
        self.in_names, self.out_names = in_names, out_names
        self.out_avals, self.zero_outs = out_avals, zero_outs
        n_params, n_outs = len(in_names), len(out_names)
        all_in = list(in_names) + list(out_names)
        if partition_name is not None:
            all_in.append(partition_name)

        def _body(*args):
            operands = list(args)
            if partition_name is not None:
                operands.append(partition_id_tensor())
            outs = _bass_exec_p.bind(
                *operands, out_avals=tuple(out_avals),
                in_names=tuple(all_in), out_names=tuple(out_names),
                lowering_input_output_aliases=(),
                sim_require_finite=True, sim_require_nnan=True, nc=nc)
            return tuple(outs)

        devices = jax.devices()[:n_cores]
        self.mesh = Mesh(np.asarray(devices), ("core",))
        self.sharding = NamedSharding(self.mesh, PartitionSpec("core"))
        in_specs = (PartitionSpec("core"),) * (n_params + n_outs)
        out_specs = (PartitionSpec("core"),) * n_outs
        self.jitted = jax.jit(
            shard_map(_body, mesh=self.mesh, in_specs=in_specs,
                      out_specs=out_specs, check_rep=False),
            keep_unused=True)
        self._dz = None

    def device_inputs(self, in_maps):
        n = self.n_cores
        concat = [
            np.concatenate(
                [np.asarray(in_maps[c][name]) for c in range(n)], axis=0)
            for name in self.in_names
        ]
        return [self.jax.device_put(a, self.sharding) for a in concat]

    def dz(self):
        if self._dz is None:
            self._dz = [
                self.jax.device_put(
                    np.zeros((self.n_cores * z.shape[0], *z.shape[1:]),
                             z.dtype), self.sharding)
                for z in self.zero_outs
            ]
        return self._dz

    def run_async(self, dev_in):
        return self.jitted(*dev_in, *self.dz())

    def run(self, dev_in):
        out_arrs = self.run_async(dev_in)
        self.jax.block_until_ready(out_arrs)
        n = self.n_cores
        return [
            {
                name: np.asarray(out_arrs[i]).reshape(
                    n, *self.out_avals[i].shape)[c]
                for i, name in enumerate(self.out_names)
            }
            for c in range(n)
        ]


_CACHE = {}


def _get_runner(inputs):
    plan1, plan2, per_core = _host_prep(inputs)
    key = (plan1.nchunks, plan2.nchunks, plan2.NROWS2,
           tuple(plan1.segs), tuple(plan2.segs))
    if _CACHE.get("key") != key:
        nc = _build(plan1, plan2)
        _CACHE["runner"] = _Runner(nc, N_CORES)
        _CACHE["key"] = key
    return _CACHE["runner"], per_core


def kernel(**inputs):
    runner, per_core = _get_runner(inputs)
    dev_in = runner.device_inputs(per_core)
    res = runner.run(dev_in)
    return np.concatenate(
        [res[c]["out_c"][:SEGN] for c in range(N_CORES)], axis=0)
